# revision 31
# baseline (speedup 1.0000x reference)
"""Trainium2 Bass kernel: MultiHeadSelfAttention (LayerNorm -> QKV -> masked
softmax attention -> output projection).

Problem shapes: B=4, S=2048, D=512, H=8, DK=64, fp32 I/O.

Sharding: 8 cores = 4 batches x 2 query-halves. Each core computes the full
K/V for its batch and attention outputs for its 1024 queries; no cross-core
communication. SPMD trick: the token order of each core's input is permuted on
the host so that the core's queries are always tokens 0..1023 (one static
program for all cores; attention is permutation-equivariant over keys as long
as the key-padding mask is permuted consistently).

Performance structure (v3): the TRN2 PE clock ramps to 2.4 GHz only after
~3us of continuous execution and any stalled instruction drops it back to
1.2 GHz, so the whole kernel is one software-pipelined stream in which the
PE is strictly the busiest engine and every PE dependency is satisfied well
before the instruction arrives:

  - Weights arrive pre-cast bf16 from the host (half the DMA, no cast ops).
  - LayerNorm on DVE; PSUM->SBUF evacuations of the transposes run on ACT.
  - Scores: one [64,128]x[64,1024] matmul per (chunk, head) into a
    [128,1024] fp32 PSUM tile (2-deep pool); one ACT op per head fuses
    scale+mask-bias+exp over all 1024 queries (wide ops amortize ACT
    overhead).  Score matmuls are interleaved with PV/filler work so the
    PSUM-recycle dependency (exp of 2 tiles ago) resolves ~600ns early.
  - PV: the exp'd scores chunk is the stationary operand, [V_h|1] the
    moving one; 16 small matmuls per chunk accumulate all 8 query tiles x
    2 heads into 3 packed PSUM banks; row 64 of each region accumulates
    the softmax denominator l.  Normalization at evacuation is then a
    per-partition (per-query) scalar multiply - cheap DVE ops.
  - Filler units keep the PE above the ACT rate in every chunk: V
    projections (pair 0), Q/K of pair p+1 and evac+transpose of pair p-1
    (pairs 1-3), out-projection partials over d-chunks 0..2 (pair 3).
    Only the last out-proj d-chunk + bias add + DMA remain in the tail.
"""

import math
from collections import deque

import ml_dtypes
import numpy as np

import concourse.bass as bass
import concourse.tile as tile
from concourse import bacc, mybir
from concourse.bass_utils import run_bass_kernel_spmd
from concourse.masks import make_identity

B, S, D, H, DK = 4, 2048, 512, 8, 64
P = 128                 # partitions
NQ = 1024               # queries per core
NT = S // P             # 16 token tiles / key chunks
DC = D // P             # 4 d-chunks
NQT = NQ // P           # 8 query tiles
PAIRS = H // 2          # 4 head pairs
QH = NQ // 512          # 2 query halves of 512
F32 = mybir.dt.float32
BF16 = mybir.dt.bfloat16
NEG = -1.0e30


def _emit(tc: tile.TileContext, ctx):
    nc = tc.nc

    x_d = nc.dram_tensor("x", [S, D], F32, kind="ExternalInput")
    wq_d = nc.dram_tensor("wq", [D, D], BF16, kind="ExternalInput")
    wk_d = nc.dram_tensor("wk", [D, D], BF16, kind="ExternalInput")
    wv_d = nc.dram_tensor("wv", [D, D], BF16, kind="ExternalInput")
    wo_d = nc.dram_tensor("wo", [D, D], BF16, kind="ExternalInput")
    bq_d = nc.dram_tensor("bq", [P, DC], F32, kind="ExternalInput")
    bk_d = nc.dram_tensor("bk", [P, DC], F32, kind="ExternalInput")
    bo_d = nc.dram_tensor("bo", [D], F32, kind="ExternalInput")
    mb_d = nc.dram_tensor("maskb", [P, NT], F32, kind="ExternalInput")
    y_d = nc.dram_tensor("y", [NQ, D], F32, kind="ExternalOutput")

    consts = ctx.enter_context(tc.tile_pool(name="consts", bufs=1))
    big = ctx.enter_context(tc.tile_pool(name="big", bufs=1))
    xstage = ctx.enter_context(tc.tile_pool(name="xstage", bufs=4))
    xnp = ctx.enter_context(tc.tile_pool(name="xnp", bufs=3))
    stats = ctx.enter_context(tc.tile_pool(name="stats", bufs=12))
    ptp = ctx.enter_context(tc.tile_pool(name="ptp", bufs=4))
    rlp = ctx.enter_context(tc.tile_pool(name="rlp", bufs=6))
    yout = ctx.enter_context(tc.tile_pool(name="yout", bufs=3))

    ident = consts.tile([P, P], BF16, tag="ident")
    make_identity(nc, ident)
    bq_sb = consts.tile([P, DC], F32, tag="bq")
    nc.sync.dma_start(bq_sb, bq_d[:, :])
    bk_sb = consts.tile([P, DC], F32, tag="bk")
    nc.sync.dma_start(bk_sb, bk_d[:, :])
    mb_sb = consts.tile([P, NT], F32, tag="mb")
    nc.sync.dma_start(mb_sb, mb_d[:, :])
    eps_sb = consts.tile([P, 1], F32, tag="eps")
    nc.vector.memset(eps_sb, 1e-5)
    bo_sb = consts.tile([P, D], F32, tag="bo")
    bo_ap = bo_d[:]
    nc.sync.dma_start(
        bo_sb, bass.AP(tensor=bo_ap.tensor, offset=bo_ap.offset, ap=[[0, P], [1, D]])
    )

    # DMA plan: quarter-split every big transfer so it spreads across DMA
    # queues (a [128,512] f32 tile is ~11us on one queue), and interleave
    # x tiles with the weight loads in the order the pipeline consumes them:
    # x0-3, wq, x4-7, wk, x8-11, wv, x12-15, wo.
    w_sb = {}
    for name in ("wq", "wk", "wv", "wo"):
        w_sb[name] = big.tile([P, DC, D], BF16, tag=f"{name}_sb", name=f"{name}_sb")
    xts = [
        xstage.tile([P, D], F32, tag="xstage", name=f"xt{t}") for t in range(NT)
    ]

    def dma_x(t):
        # split by partition rows: 4 parallel DMAs, 2KB-contiguous lines
        for i in range(4):
            nc.sync.dma_start(
                xts[t][i * 32 : (i + 1) * 32, :],
                x_d[t * P + i * 32 : t * P + (i + 1) * 32, :],
            )

    def dma_w(name, d):
        for c in range(DC):
            for i in range(2):
                nc.sync.dma_start(
                    w_sb[name][i * 64 : (i + 1) * 64, c, :],
                    d[c * P + i * 64 : c * P + (i + 1) * 64, :],
                )

    for t in range(4):
        dma_x(t)
    dma_w("wq", wq_d)
    for t in range(4, 8):
        dma_x(t)
    dma_w("wk", wk_d)
    for t in range(8, 12):
        dma_x(t)
    dma_w("wv", wv_d)
    for t in range(12, NT):
        dma_x(t)
    dma_w("wo", wo_d)
    xnT = big.tile([P, DC, S], BF16, tag="xnT")
    qT = big.tile([P, DC, NQ], BF16, tag="qT")      # d-chunk == pair
    kT = big.tile([P, DC, S], BF16, tag="kT")       # d-chunk == pair
    vaug = big.tile([P, NT, H * 65], BF16, tag="vaug")  # [V_h | 1] per head
    attno = big.tile([P, NQT, D], BF16, tag="attno")    # token-major attn out
    outT = big.tile([P, DC, NQ], BF16, tag="outT")      # transposed attn out
    osum = big.tile([P, NQT, D], F32, tag="osum")   # out-proj partials + bias

    # ---------------- filler units (emitted interleaved into attention) ----
    # Each unit is ~0.2-0.9us of PE work plus its evacuation; projS (one
    # PSUM bank, bufs=1) serializes consecutive units.
    projS_ref = [None]

    def unit_v(t):
        def emit():
            projS = projS_ref[0]
            po = projS.tile([P, D], F32, tag="ps", name=f"v{t}")
            for dc in range(DC):
                nc.tensor.matmul(
                    po,
                    xnT[:, dc, t * P : (t + 1) * P],
                    w_sb["wv"][:, dc, :],
                    start=(dc == 0), stop=(dc == DC - 1),
                )
            vslot = vaug[:, t, :].rearrange("p (h c) -> p h c", h=H)
            nc.vector.tensor_copy(
                out=vslot[:, :, 0:DK],
                in_=po[:].rearrange("p (h c) -> p h c", h=H),
            )
            nc.vector.memset(vslot[:, :, DK : DK + 1], 1.0)
        return emit

    def unit_q(p, qh):
        def emit():
            projS = projS_ref[0]
            ps = projS.tile([P, D], F32, tag="ps", name=f"q{p}_{qh}")
            for dc in range(DC):
                nc.tensor.matmul(
                    ps,
                    w_sb["wq"][:, dc, p * P : (p + 1) * P],
                    xnT[:, dc, qh * 512 : (qh + 1) * 512],
                    start=(dc == 0), stop=(dc == DC - 1),
                )
            nc.vector.tensor_scalar_add(
                out=qT[:, p, qh * 512 : (qh + 1) * 512], in0=ps,
                scalar1=bq_sb[:, p : p + 1],
            )
        return emit

    def unit_k(p, kg):
        def emit():
            projS = projS_ref[0]
            ps = projS.tile([P, D], F32, tag="ps", name=f"k{p}_{kg}")
            for dc in range(DC):
                nc.tensor.matmul(
                    ps,
                    w_sb["wk"][:, dc, p * P : (p + 1) * P],
                    xnT[:, dc, kg * 512 : (kg + 1) * 512],
                    start=(dc == 0), stop=(dc == DC - 1),
                )
            nc.vector.tensor_scalar_add(
                out=kT[:, p, kg * 512 : (kg + 1) * 512], in0=ps,
                scalar1=bk_sb[:, p : p + 1],
            )
        return emit

    def unit_evac(p, pvb, qt):
        # normalize the packed PV accumulators of query-tile qt by 1/l
        # (per-partition scalars) into token-major attno.
        def emit():
            bank = pvb[qt // 3]
            off = (qt % 3) * 130
            rl = rlp.tile([P, 2], F32, tag="rl")
            for hs in range(2):
                nc.vector.reciprocal(
                    out=rl[:, hs : hs + 1],
                    in_=bank[:, off + hs * 65 + DK : off + hs * 65 + DK + 1],
                )
            for hs in range(2):
                nc.vector.tensor_scalar_mul(
                    out=attno[:, qt, (2 * p + hs) * DK : (2 * p + hs + 1) * DK],
                    in0=bank[:, off + hs * 65 : off + hs * 65 + DK],
                    scalar1=rl[:, hs : hs + 1],
                )
        return emit

    def unit_t(p, g):
        # transpose pair p's attno columns for query tiles 4g..4g+3 into outT
        def emit():
            projS = projS_ref[0]
            pe = projS.tile([P, D], BF16, tag="ps", name=f"t{p}_{g}")
            for i in range(4):
                qt = 4 * g + i
                nc.tensor.transpose(
                    pe[:, i * P : (i + 1) * P],
                    attno[:, qt, p * P : (p + 1) * P], ident,
                )
            nc.vector.tensor_copy(
                out=outT[:, p, g * 512 : (g + 1) * 512], in_=pe
            )
        return emit

    def unit_opj(qt):
        # out-projection partial: d-chunks 0..2 + bias, stashed f32
        def emit():
            projS = projS_ref[0]
            po = projS.tile([P, D], F32, tag="ps", name=f"opj{qt}")
            for dc in range(DC - 1):
                nc.tensor.matmul(
                    po,
                    outT[:, dc, qt * P : (qt + 1) * P],
                    w_sb["wo"][:, dc, :],
                    start=(dc == 0), stop=(dc == DC - 2),
                )
            nc.vector.tensor_tensor(
                out=osum[:, qt, :], in0=po, in1=bo_sb, op=mybir.AluOpType.add
            )
        return emit

    # ---------------- front: LN + transposes + Q/K(pair0) + V(0,1) ---------
    with tc.tile_pool(name="projF", bufs=4, space="PSUM") as projF:
        projS_ref[0] = projF
        for t in range(NT):
            xt = xts[t]
            st = stats.tile([P, 6], F32, tag="st")
            nc.vector.bn_stats(out=st, in_=xt)
            mv = stats.tile([P, 2], F32, tag="mv")
            nc.vector.bn_aggr(out=mv, in_=st)
            sd = stats.tile([P, 1], F32, tag="sd")
            nc.scalar.activation(
                out=sd, in_=mv[:, 1:2], func=mybir.ActivationFunctionType.Sqrt,
                bias=eps_sb,
            )
            rr = stats.tile([P, 1], F32, tag="rr")
            nc.vector.reciprocal(out=rr, in_=sd)
            xn = xnp.tile([P, D], BF16, tag="xn")
            nc.vector.tensor_scalar(
                out=xn, in0=xt, scalar1=mv[:, 0:1], scalar2=rr,
                op0=mybir.AluOpType.subtract, op1=mybir.AluOpType.mult,
            )
            pt4 = projF.tile([P, D], BF16, tag="tp")
            for c in range(DC):
                nc.tensor.transpose(
                    pt4[:, c * P : (c + 1) * P], xn[:, c * P : (c + 1) * P], ident
                )
            # PSUM->SBUF evacuation on ACT to keep DVE free for bn_stats
            nc.scalar.activation(
                out=xnT[:, :, t * P : (t + 1) * P],
                in_=pt4[:].rearrange("p (c q) -> p c q", c=DC),
                func=mybir.ActivationFunctionType.Copy,
            )
            if t == NQT - 1:
                for qh in range(QH):
                    unit_q(0, qh)()
        for kg in range(QH):
            unit_k(0, kg)()
        unit_v(0)()
        for kg in range(QH, S // 512):
            unit_k(0, kg)()
        unit_v(1)()

    # ---------------- attention: pairs x chunks, software-pipelined --------
    with (
        tc.tile_pool(name="scp", bufs=2, space="PSUM") as scp,
        tc.tile_pool(name="pvp", bufs=3, space="PSUM") as pvp,
        tc.tile_pool(name="projS", bufs=1, space="PSUM") as projS,
    ):
        projS_ref[0] = projS

        def filler_for(p, pvb_prev):
            f = deque()
            n_evac = 0
            if pvb_prev is not None:
                for qt in range(NQT):
                    f.append(unit_evac(p - 1, pvb_prev, qt))
                n_evac = NQT
            if p == 0:
                for t in range(2, NT):
                    f.append(unit_v(t))
            if p >= 1:
                for g in range(QH):
                    f.append(unit_t(p - 1, g))
            if p < PAIRS - 1:
                for qh in range(QH):
                    f.append(unit_q(p + 1, qh))
                for kg in range(S // 512):
                    f.append(unit_k(p + 1, kg))
            if p == PAIRS - 1:
                for qt in range(NQT):
                    f.append(unit_opj(qt))
            return f, n_evac

        pvb_prev = None
        for p in range(PAIRS):
            pvb = [
                pvp.tile([P, 512], F32, tag="pvb", name=f"pvb{p}_{j}")
                for j in range(3)
            ]
            filler, n_evac = filler_for(p, pvb_prev)
            pts = {}
            for c in range(NT):
                # per-chunk PE order: sc(h0) | fillers | PV half | sc(h1) |
                # fillers | PV half -- scores interleaved with independent
                # work so their PSUM-recycle waits resolve early, and the
                # previous pair's 8 evac units all land before PV(c=0).
                if n_evac > 0:
                    n = 4
                    n_evac -= 4
                else:
                    n = min(3, -(-len(filler) // (NT - c)))
                for hs in range(2):
                    sc = scp.tile([P, NQ], F32, tag="sc")
                    for qh in range(QH):
                        nc.tensor.matmul(
                            sc[:, qh * 512 : (qh + 1) * 512],
                            kT[hs * DK : (hs + 1) * DK, p, c * P : (c + 1) * P],
                            qT[hs * DK : (hs + 1) * DK, p, qh * 512 : (qh + 1) * 512],
                            start=True, stop=True,
                        )
                    pt = ptp.tile([P, NQ], BF16, tag="pt")
                    nc.scalar.activation(
                        out=pt, in_=sc,
                        func=mybir.ActivationFunctionType.Exp,
                        bias=mb_sb[:, c : c + 1], scale=1.0 / math.sqrt(DK),
                    )
                    pts[(c, hs)] = pt
                    for _ in range((n + 1 - hs) // 2):  # split fillers h0/h1
                        if filler:
                            filler.popleft()()
                    if c > 0:
                        _pv_half(nc, pvb, pts, vaug, p, c - 1, hs)
            while filler:
                filler.popleft()()
            for hs in range(2):
                _pv_half(nc, pvb, pts, vaug, p, NT - 1, hs)
            pts.clear()
            pvb_prev = pvb

        # ---------------- tail: evac pair 3, transpose, final out-proj ----
        for qt in range(NQT):
            unit_evac(PAIRS - 1, pvb_prev, qt)()
            if qt % 4 == 3:
                unit_t(PAIRS - 1, qt // 4)()
                for q2 in range(qt - 3, qt + 1):
                    po = projS.tile([P, D], F32, tag="ps", name=f"fin{q2}")
                    nc.tensor.matmul(
                        po,
                        outT[:, DC - 1, q2 * P : (q2 + 1) * P],
                        w_sb["wo"][:, DC - 1, :],
                        start=True, stop=True,
                    )
                    yt = yout.tile([P, D], F32, tag="yt")
                    nc.vector.tensor_tensor(
                        out=yt, in0=po, in1=osum[:, q2, :], op=mybir.AluOpType.add
                    )
                    for i in range(4):
                        nc.sync.dma_start(
                            y_d[q2 * P + i * 32 : q2 * P + (i + 1) * 32, :],
                            yt[i * 32 : (i + 1) * 32, :],
                        )


def _pv_half(nc, pvb, pts, vaug, p, c, hs):
    """PV matmuls for (key-chunk c, head hs) of pair p: the exp'd scores
    chunk is the stationary operand, [V_h|1] the moving one; all 8 query
    tiles accumulate into the 3 packed PSUM banks (region (qt%3)*130+hs*65,
    denominator l in its column 64)."""
    pt = pts[(c, hs)]
    h = 2 * p + hs
    for qt in range(NQT):
        bank = pvb[qt // 3]
        off = (qt % 3) * 130 + hs * 65
        # start=True clears has_written for the WHOLE bank, so only the
        # first packed region per bank may use it; the others rely on
        # overwrite-when-bit-clear for their first chunk.
        nc.tensor.matmul(
            bank[:, off : off + 65],
            pt[:, qt * P : (qt + 1) * P],
            vaug[:, c, h * 65 : (h + 1) * 65],
            start=(c == 0 and qt % 3 == 0 and hs == 0),
            stop=(c == NT - 1),
            skip_group_check=True,
        )


_NC = None


def _get_nc():
    global _NC
    if _NC is None:
        from contextlib import ExitStack

        nc = bacc.Bacc(None, target_bir_lowering=False)
        with tile.TileContext(nc) as tc, ExitStack() as ctx:
            _emit(tc, ctx)
        nc.compile()
        _NC = nc
    return _NC


def kernel(
    inputs, input_lengths, pos_embed, ln_gamma, ln_beta,
    Wq, bq, Wk, bk, Wv, bv, Wo, bo,
):
    x = np.ascontiguousarray(np.asarray(inputs, np.float32))
    lengths = np.asarray(input_lengths, np.int32)
    g = np.asarray(ln_gamma, np.float32)
    be = np.asarray(ln_beta, np.float32)
    Wq = np.asarray(Wq, np.float32); bq = np.asarray(bq, np.float32)
    Wk = np.asarray(Wk, np.float32); bk = np.asarray(bk, np.float32)
    Wv = np.asarray(Wv, np.float32); bv = np.asarray(bv, np.float32)
    Wo = np.asarray(Wo, np.float32); bo = np.asarray(bo, np.float32)

    # Fold LayerNorm affine into the projections (exact: LN(x) = xh*g + be
    # with xh = (x-mu)*rstd, so LN(x)@W.T + b = xh@(g[:,None]*W.T) + (be@W.T + b)).
    bf16 = ml_dtypes.bfloat16
    wq_h = np.ascontiguousarray((g[:, None] * Wq.T).astype(bf16))
    wk_h = np.ascontiguousarray((g[:, None] * Wk.T).astype(bf16))
    wv_h = np.ascontiguousarray((g[:, None] * Wv.T).astype(bf16))
    wo_h = np.ascontiguousarray(Wo.T.astype(bf16))
    bq_h = np.ascontiguousarray((be @ Wq.T + bq).reshape(DC, P).T)
    bk_h = np.ascontiguousarray((be @ Wk.T + bk).reshape(DC, P).T)
    # V bias (incl. beta term) passes through softmax (rows sum to 1) and is
    # folded into the output-projection bias.
    bv_h = be @ Wv.T + bv
    bo_h = np.ascontiguousarray(bo + bv_h @ Wo.T)

    maskb = np.where(np.arange(S)[None, :] < lengths[:, None], 0.0, NEG).astype(
        np.float32
    )

    nc = _get_nc()
    in_maps = []
    core_assign = []
    for b in range(B):
        for gq in range(2):
            order = np.r_[gq * NQ : (gq + 1) * NQ, (1 - gq) * NQ : (2 - gq) * NQ]
            in_maps.append(
                {
                    "x": np.ascontiguousarray(x[b][order]),
                    "wq": wq_h, "wk": wk_h, "wv": wv_h, "wo": wo_h,
                    "bq": bq_h, "bk": bk_h, "bo": bo_h,
                    "maskb": np.ascontiguousarray(maskb[b][order].reshape(NT, P).T),
                }
            )
            core_assign.append((b, gq))

    global _LAST_IN_MAPS
    _LAST_IN_MAPS = in_maps
    res = run_bass_kernel_spmd(nc, in_maps, core_ids=list(range(8)))

    y = np.empty((B, S, D), np.float32)
    for i, (b, gq) in enumerate(core_assign):
        y[b, gq * NQ : (gq + 1) * NQ] = res.results[i]["y"]
    return y
